# revision 12
# baseline (speedup 1.0000x reference)
"""Trainium2 Bass kernel for nn_Attention (dense transformer attention).

Math (per batch n, head h):
  q' = q_h @ Wq.T ; k' = k_h @ Wk.T ; v' = v_h @ Wv.T
  S = (q' k'^T)/32 ; P = exp(S) ; out_h = (P v') / rowsum(P)
  final = concat_h(out_h) @ Wout.T + bout

Host-side folding (exact in real arithmetic, bf16-rounded once):
  S   = Q'' @ K^T        with Q'' = Q @ (Wq.T @ Wk)/32   (folded on host)
  V'  = V @ Wv.T                                          (folded on host)
so the device only does: scores -> exp -> [V'|1]-weighted sums ->
normalize -> fc_out.

Sharding: 8 cores = 2 batches x 4 query blocks of 512. Each core reads its
batch's K^T / packed V' plus its 512-query slice of Q''^T and writes its
[1024, 512] slice of final^T. No collectives; host concatenates.

Device layouts (host-prepped so every DMA is contiguous):
  kT   (2, 1024, 2048) bf16   K^T (embed-major)
  qT   per-core (1024, 512) bf16   Q''^T slice
  vp   per-core (8, 128, 16, 130) bf16  per head-pair packed
        [V'_h0 (64) | 1 | V'_h1 (64) | 1] per (token%128, chunk)
  wout (128, 8, 1024) bf16    Wout.T rearranged (ec p) o -> p ec o
  bias (128, 8) f32           bout.reshape(8,128).T
  ones (128, 128) f32         broadcast helper
Output: outT (1024, 512) f32 = final^T slice.

Per head-pair device flow (heads 2e, 2e+1 share partitions 0-63 / 64-127):
  - scores: 16 chunks; two row-group-paired matmuls (stationary = kT chunk
    rows 0-63 / 64-127) -> PSUM [128, 2, 512] fp32
  - exp: one ACT instr per chunk [128, 2, 512] PSUM -> SBUF bf16
  - PV: per chunk, per head: stationary [128 tok, 65] = [V'_h | 1],
    accumulate into PSUM U [65, 512]: rows 0-63 = U'_h, row 64 = denom
  - normalize: recip(denom) -> PE outer-product broadcast to 64 rows ->
    DVE multiply -> attn tile (head1 via bounce + partition-shift DMA)
  - fc_out: 8x8 [128,128] bf16 matmuls accumulating over head-pairs
"""

import sys

for p in ("/opt/trn_rl_repo",):
    if p not in sys.path:
        sys.path.insert(0, p)

import numpy as np
import ml_dtypes

BF16 = ml_dtypes.bfloat16

N = 2
L = 2048
E = 1024
H = 16
D = 64
NCORES = 8
NQBLK = 4                 # query blocks per batch
LQ = L // NQBLK           # 512 queries per core
NPAIR = H // 2            # 8 head-pairs
NCHUNK = L // 128         # 16 key chunks of 128 tokens
import os as _os
REPEAT = int(_os.environ.get("BASS_KERNEL_REPEAT", "1"))


def build_nc():
    import concourse.bass as bass
    import concourse.bacc as bacc
    import concourse.mybir as mybir
    import concourse.tile as tile

    f32 = mybir.dt.float32
    f32r = mybir.dt.float32r
    bf16 = mybir.dt.bfloat16
    EXP = mybir.ActivationFunctionType.Exp
    MUL = mybir.AluOpType.mult
    ADD = mybir.AluOpType.add

    nc = bacc.Bacc(None, target_bir_lowering=False)

    kT = nc.dram_tensor("kT", [E, L], bf16, kind="ExternalInput")
    qT = nc.dram_tensor("qT", [E, LQ], bf16, kind="ExternalInput")
    vp = nc.dram_tensor("vp", [NPAIR, 128, NCHUNK, 130], bf16, kind="ExternalInput")
    wout = nc.dram_tensor("wout", [128, E // 128, E], bf16, kind="ExternalInput")
    bias = nc.dram_tensor("bias", [128, E // 128], f32, kind="ExternalInput")
    ones = nc.dram_tensor("ones", [128, 128], f32r, kind="ExternalInput")
    outT = nc.dram_tensor("outT", [E, LQ], f32, kind="ExternalOutput")

    with tile.TileContext(nc) as tc:
        with (
            tc.tile_pool(name="const", bufs=1) as const,
            tc.tile_pool(name="io", bufs=2) as io,
            tc.tile_pool(name="exps", bufs=2) as exps_pool,
            tc.tile_pool(name="work", bufs=3) as work,
            tc.tile_pool(name="attn", bufs=1) as attn_pool,
        ):
            # --- persistent constants (wout is streamed in per-pair chunks
            # inside the loop so it doesn't delay the first pair's loads) ---
            wout_sb = const.tile([128, E // 128, E], bf16)
            bias_sb = const.tile([128, E // 128], f32)
            nc.scalar.dma_start(bias_sb, bias[:, :])
            ones_sb = const.tile([128, 128], f32r)
            nc.scalar.dma_start(ones_sb, ones[:, :])

            import contextlib

            rep_ctx = (
                tc.For_i(0, REPEAT, 1) if REPEAT > 1 else contextlib.nullcontext()
            )
            with rep_ctx:
                attn_sb = attn_pool.tile([128, NPAIR, LQ], bf16, tag="attn")
                with (
                    tc.tile_pool(name="psT", bufs=2, space="PSUM") as psT,
                    tc.tile_pool(name="puT", bufs=1, space="PSUM") as puT,
                    tc.tile_pool(name="pb", bufs=1, space="PSUM") as pbp,
                ):
                    for e in range(NPAIR):
                        # --- loads for this head pair ---
                        kT2 = io.tile([128, L], bf16, tag="kT2")
                        nc.sync.dma_start(kT2, kT[128 * e : 128 * (e + 1), :])
                        q2 = io.tile([128, LQ], bf16, tag="q2")
                        nc.sync.dma_start(q2, qT[128 * e : 128 * (e + 1), :])
                        v2 = io.tile([128, NCHUNK, 130], bf16, tag="v2")
                        nc.sync.dma_start(v2, vp[e])
                        nc.scalar.dma_start(wout_sb[:, e, :], wout[:, e, :])

                        expS = exps_pool.tile(
                            [128, 2, NCHUNK, LQ], bf16, tag="expS"
                        )
                        uT0 = puT.tile([65, LQ], f32, tag="uT0")
                        uT1 = puT.tile([65, LQ], f32, tag="uT1")
                        for ch in range(NCHUNK):
                            # scores: both heads via disjoint PE row groups
                            sT = psT.tile([128, 2, LQ], f32, tag="sT")
                            for hh in range(2):
                                nc.tensor.matmul(
                                    sT[:, hh, :],
                                    kT2[64 * hh : 64 * hh + 64,
                                        128 * ch : 128 * (ch + 1)],
                                    q2[64 * hh : 64 * hh + 64, :],
                                    start=True, stop=True,
                                )
                            nc.scalar.activation(
                                expS[:, :, ch, :], sT[:, :, :], EXP
                            )
                            # PV: accumulate [V'|1]^T @ expS^T per head
                            nc.tensor.matmul(
                                uT0,
                                v2[:, ch, 0:65],
                                expS[:, 0, ch, :],
                                start=(ch == 0), stop=(ch == NCHUNK - 1),
                            )
                            nc.tensor.matmul(
                                uT1,
                                v2[:, ch, 65:130],
                                expS[:, 1, ch, :],
                                start=(ch == 0), stop=(ch == NCHUNK - 1),
                            )

                        # --- normalize ---
                        # copy U out of PSUM quickly (frees uT for next pair)
                        u_sb = work.tile([65, 2, LQ], f32r, tag="u_sb")
                        nc.vector.tensor_copy(u_sb[:, 0, :], uT0)
                        nc.vector.tensor_copy(u_sb[:, 1, :], uT1)
                        r_sb = work.tile([65, 2, LQ], f32r, tag="r_sb")
                        with nc.allow_low_precision("feeds f32r bcast matmul"):
                            nc.vector.reciprocal(
                                r_sb[64:65, :, :], u_sb[64:65, :, :]
                            )
                        # broadcast 1/denom across 64 partitions via PE outer
                        pb = pbp.tile([64, 2, LQ], f32, tag="pb")
                        for hh in range(2):
                            nc.tensor.matmul(
                                pb[:, hh, :],
                                ones_sb[64:65, 0:64],
                                r_sb[64:65, hh, :],
                                start=True, stop=True,
                            )
                        nc.vector.tensor_tensor(
                            attn_sb[0:64, e, :], u_sb[0:64, 0, :], pb[:, 0, :], MUL
                        )
                        bounce = work.tile([64, LQ], bf16, tag="bounce")
                        nc.vector.tensor_tensor(
                            bounce, u_sb[0:64, 1, :], pb[:, 1, :], MUL
                        )
                        nc.sync.dma_start(attn_sb[64:128, e, :], bounce)

                # --- fc_out: final^T = Wout @ attn^T + bout ---
                with tc.tile_pool(name="po", bufs=2, space="PSUM") as pop:
                    for oc in range(E // 128):
                        po = pop.tile([128, LQ], f32, tag="po")
                        for ec in range(E // 128):
                            nc.tensor.matmul(
                                po,
                                wout_sb[:, ec, 128 * oc : 128 * (oc + 1)],
                                attn_sb[:, ec, :],
                                start=(ec == 0), stop=(ec == E // 128 - 1),
                            )
                        o_sb = work.tile([128, LQ], f32, tag="o_sb")
                        nc.vector.tensor_tensor(
                            o_sb, po,
                            bias_sb[:, oc : oc + 1].to_broadcast((128, LQ)),
                            ADD,
                        )
                        nc.sync.dma_start(
                            outT[128 * oc : 128 * (oc + 1), :], o_sb
                        )

    nc.compile()
    return nc


def shard_inputs(values, keys, query, Wv, Wk, Wq, Wout, bout):
    f = np.float32
    values = np.asarray(values, dtype=f)
    keys = np.asarray(keys, dtype=f)
    query = np.asarray(query, dtype=f)
    Wv, Wk, Wq, Wout, bout = (np.asarray(x, dtype=f) for x in (Wv, Wk, Wq, Wout, bout))

    # fold projections on host
    Wc = (Wq.T @ Wk) / np.float32(np.sqrt(E))
    q4 = query.reshape(N, L, H, D) @ Wc          # Q'' per head
    v4 = values.reshape(N, L, H, D) @ Wv.T       # V' per head

    # K^T (embed-major), bf16
    kT = np.ascontiguousarray(keys.transpose(0, 2, 1)).astype(BF16)
    # Q''^T, bf16
    qT = np.ascontiguousarray(
        q4.reshape(N, L, E).transpose(0, 2, 1)
    ).astype(BF16)
    # packed V': [n, pair, token%128, chunk, 130] = [V'_h0 | 1 | V'_h1 | 1]
    vp = np.ones((N, NPAIR, 128, NCHUNK, 130), dtype=f)
    # v4 -> [n, chunk, part, pair, head%2, d]
    v6 = v4.reshape(N, NCHUNK, 128, NPAIR, 2, D)
    vp[:, :, :, :, 0:64] = v6[:, :, :, :, 0].transpose(0, 3, 2, 1, 4)
    vp[:, :, :, :, 65:129] = v6[:, :, :, :, 1].transpose(0, 3, 2, 1, 4)
    vp = vp.astype(BF16)

    wout_r = np.ascontiguousarray(
        Wout.T.reshape(E // 128, 128, E).transpose(1, 0, 2)
    ).astype(BF16)
    bias2 = np.ascontiguousarray(bout.reshape(E // 128, 128).T, dtype=f)
    ones = np.ones((128, 128), dtype=f)

    in_maps = []
    for c in range(NCORES):
        n, qb = c // NQBLK, c % NQBLK
        in_maps.append({
            "kT": kT[n],
            "qT": np.ascontiguousarray(qT[n, :, qb * LQ : (qb + 1) * LQ]),
            "vp": vp[n],
            "wout": wout_r,
            "bias": bias2,
            "ones": ones,
        })
    return in_maps


def unshard(results):
    out = np.empty((N, L, E), dtype=np.float32)
    for c, r in enumerate(results):
        n, qb = c // NQBLK, c % NQBLK
        out[n, qb * LQ : (qb + 1) * LQ, :] = np.asarray(r["outT"]).T
    return out


def run_spmd(in_maps, **kwargs):
    from concourse.bass_utils import run_bass_kernel_spmd

    nc = build_nc()
    res = run_bass_kernel_spmd(nc, in_maps, core_ids=list(range(NCORES)), **kwargs)
    return nc, res


def kernel(**inputs):
    in_maps = shard_inputs(
        inputs["values"], inputs["keys"], inputs["query"],
        inputs["Wv"], inputs["Wk"], inputs["Wq"],
        inputs["Wout"], inputs["bout"],
    )
    _, res = run_spmd(in_maps)
    return unshard(res.results)


if __name__ == "__main__":
    rng = np.random.default_rng(0)
    ins = {
        "values": rng.standard_normal((N, L, E), dtype=np.float32),
        "keys": rng.standard_normal((N, L, E), dtype=np.float32),
        "query": rng.standard_normal((N, L, E), dtype=np.float32),
        "Wv": rng.standard_normal((D, D), dtype=np.float32) / 8,
        "Wk": rng.standard_normal((D, D), dtype=np.float32) / 8,
        "Wq": rng.standard_normal((D, D), dtype=np.float32) / 8,
        "Wout": rng.standard_normal((E, E), dtype=np.float32) / 32,
        "bout": rng.standard_normal((E,), dtype=np.float32) * 0.01,
    }
    out = kernel(**ins)
    print("out", out.shape, out.dtype, float(np.abs(out).max()))


# revision 15
# speedup vs baseline: 5.0922x; 5.0922x over previous
"""Trainium2 Bass kernel for nn_Attention (dense transformer attention).

Math (per batch n, head h):
  q' = q_h @ Wq.T ; k' = k_h @ Wk.T ; v' = v_h @ Wv.T
  S = (q' k'^T)/32 ; P = exp(S) ; out_h = (P v') / rowsum(P)
  final = concat_h(out_h) @ Wout.T + bout

Host-side folding (exact in real arithmetic, bf16-rounded once):
  S   = Q'' @ K^T        with Q'' = Q @ (Wq.T @ Wk)/32   (folded on host)
  V'  = V @ Wv.T                                          (folded on host)
so the device only does: scores -> exp -> [V'|1]-weighted sums ->
normalize -> fc_out.

Sharding: 8 cores = 2 batches x 4 query blocks of 512. Each core reads its
batch's K^T / packed V' plus its 512-query slice of Q''^T and writes its
[1024, 512] slice of final^T. No collectives; host concatenates.

Device layouts (host-prepped so every DMA is contiguous):
  kT   (1024, 2048) bf16      K^T slice (embed-major), loaded in halves
  qT   (1024, 512)  bf16      Q''^T slice
  vp   (8, 128, 16, 130) bf16 per head-pair packed
        [V'_h0 (64) | 1 | V'_h1 (64) | 1] per (token%128, chunk)
  wout (128, 8, 1024) bf16    Wout.T rearranged (ec p) o -> p ec o,
                              streamed in per-pair chunks during the loop
  bias (128, 8) f32           bout.reshape(8,128).T
  ones (128, 128) f32         broadcast helper
Output: outT (1024, 512) f32 = final^T slice.

Per head-pair device flow (heads 2e, 2e+1 share partitions 0-63 / 64-127):
  - scores: 16 chunks; two row-group-paired matmuls (stationary = kT chunk
    rows 0-63 / 64-127) -> PSUM [128, 2, 512] fp32
  - exp: one ACT instr per chunk [128, 2, 512] PSUM -> SBUF bf16 (ACT is
    the bottleneck engine: ~134 us busy per core)
  - PV: per chunk, per head: stationary [128 tok, 65] = [V'_h | 1],
    accumulate into PSUM U [65, 512]: rows 0-63 = U'_h, row 64 = denom
  - normalize: copy U to SBUF (frees PSUM), recip(denom) -> PE
    outer-product broadcast to 64 rows -> DVE multiply -> attn tile
    (head1 via bounce + partition-shift DMA)
  - fc_out: 8x8 [128,128] bf16 matmuls accumulating over head-pairs
PSUM budget is exactly 8 banks: sT 2x[128,2,512] + uT0 + uT1 + pb [64,2,512].
"""

import sys

for p in ("/opt/trn_rl_repo",):
    if p not in sys.path:
        sys.path.insert(0, p)

import numpy as np
import ml_dtypes

BF16 = ml_dtypes.bfloat16

N = 2
L = 2048
E = 1024
H = 16
D = 64
NCORES = 8
NQBLK = 4                 # query blocks per batch
LQ = L // NQBLK           # 512 queries per core
NPAIR = H // 2            # 8 head-pairs
NCHUNK = L // 128         # 16 key chunks of 128 tokens
import os as _os
REPEAT = int(_os.environ.get("BASS_KERNEL_REPEAT", "1"))


def build_nc():
    import concourse.bass as bass
    import concourse.bacc as bacc
    import concourse.mybir as mybir
    import concourse.tile as tile

    f32 = mybir.dt.float32
    f32r = mybir.dt.float32r
    bf16 = mybir.dt.bfloat16
    EXP = mybir.ActivationFunctionType.Exp
    MUL = mybir.AluOpType.mult
    ADD = mybir.AluOpType.add

    nc = bacc.Bacc(None, target_bir_lowering=False)

    kT = nc.dram_tensor("kT", [E, L], bf16, kind="ExternalInput")
    qT = nc.dram_tensor("qT", [E, LQ], bf16, kind="ExternalInput")
    vp = nc.dram_tensor("vp", [NPAIR, 128, NCHUNK, 130], bf16, kind="ExternalInput")
    wout = nc.dram_tensor("wout", [128, E // 128, E], bf16, kind="ExternalInput")
    bias = nc.dram_tensor("bias", [128, E // 128], f32, kind="ExternalInput")
    ones = nc.dram_tensor("ones", [128, 128], f32r, kind="ExternalInput")
    outT = nc.dram_tensor("outT", [E, LQ], f32, kind="ExternalOutput")

    with tile.TileContext(nc) as tc:
        with (
            tc.tile_pool(name="const", bufs=1) as const,
            tc.tile_pool(name="io", bufs=3) as io,
            tc.tile_pool(name="exps", bufs=2) as exps_pool,
            tc.tile_pool(name="work", bufs=3) as work,
            tc.tile_pool(name="attn", bufs=1) as attn_pool,
        ):
            # --- persistent constants (wout is streamed in per-pair chunks
            # inside the loop so it doesn't delay the first pair's loads) ---
            wout_sb = const.tile([128, E // 128, E], bf16)
            bias_sb = const.tile([128, E // 128], f32)
            nc.scalar.dma_start(bias_sb, bias[:, :])
            ones_sb = const.tile([128, 128], f32r)
            nc.scalar.dma_start(ones_sb, ones[:, :])

            import contextlib

            rep_ctx = (
                tc.For_i(0, REPEAT, 1) if REPEAT > 1 else contextlib.nullcontext()
            )
            with rep_ctx:
                attn_sb = attn_pool.tile([128, NPAIR, LQ], bf16, tag="attn")
                with (
                    tc.tile_pool(name="psT", bufs=2, space="PSUM") as psT,
                    tc.tile_pool(name="puT", bufs=1, space="PSUM") as puT,
                    tc.tile_pool(name="pb", bufs=1, space="PSUM") as pbp,
                ):
                    for e in range(NPAIR):
                        # --- loads for this head pair (kT/v split in halves
                        # so chunk-0 compute starts before the full load) ---
                        kT2 = io.tile([128, L], bf16, tag="kT2")
                        q2 = io.tile([128, LQ], bf16, tag="q2")
                        nc.sync.dma_start(q2, qT[128 * e : 128 * (e + 1), :])
                        v2 = io.tile([128, NCHUNK, 130], bf16, tag="v2")
                        for half in range(2):
                            hs = slice(half * (L // 2), (half + 1) * (L // 2))
                            nc.sync.dma_start(
                                kT2[:, hs], kT[128 * e : 128 * (e + 1), hs]
                            )
                            cs = slice(half * (NCHUNK // 2), (half + 1) * (NCHUNK // 2))
                            nc.sync.dma_start(v2[:, cs, :], vp[e][:, cs, :])
                        nc.scalar.dma_start(wout_sb[:, e, :], wout[:, e, :])

                        expS = exps_pool.tile(
                            [128, 2, NCHUNK, LQ], bf16, tag="expS"
                        )
                        uT0 = puT.tile([65, LQ], f32, tag="uT0")
                        uT1 = puT.tile([65, LQ], f32, tag="uT1")
                        for ch in range(NCHUNK):
                            # scores: both heads via disjoint PE row groups
                            sT = psT.tile([128, 2, LQ], f32, tag="sT")
                            for hh in range(2):
                                nc.tensor.matmul(
                                    sT[:, hh, :],
                                    kT2[64 * hh : 64 * hh + 64,
                                        128 * ch : 128 * (ch + 1)],
                                    q2[64 * hh : 64 * hh + 64, :],
                                    start=True, stop=True,
                                )
                            nc.scalar.activation(
                                expS[:, :, ch, :], sT[:, :, :], EXP
                            )
                            # PV: accumulate [V'|1]^T @ expS^T per head
                            nc.tensor.matmul(
                                uT0,
                                v2[:, ch, 0:65],
                                expS[:, 0, ch, :],
                                start=(ch == 0), stop=(ch == NCHUNK - 1),
                            )
                            nc.tensor.matmul(
                                uT1,
                                v2[:, ch, 65:130],
                                expS[:, 1, ch, :],
                                start=(ch == 0), stop=(ch == NCHUNK - 1),
                            )

                        # --- normalize ---
                        # copy U out of PSUM quickly (frees uT for next pair)
                        u_sb = work.tile([65, 2, LQ], f32r, tag="u_sb")
                        nc.vector.tensor_copy(u_sb[:, 0, :], uT0)
                        nc.vector.tensor_copy(u_sb[:, 1, :], uT1)
                        r_sb = work.tile([65, 2, LQ], f32r, tag="r_sb")
                        with nc.allow_low_precision("feeds f32r bcast matmul"):
                            nc.vector.reciprocal(
                                r_sb[64:65, :, :], u_sb[64:65, :, :]
                            )
                        # broadcast 1/denom across 64 partitions via PE outer
                        pb = pbp.tile([64, 2, LQ], f32, tag="pb")
                        for hh in range(2):
                            nc.tensor.matmul(
                                pb[:, hh, :],
                                ones_sb[64:65, 0:64],
                                r_sb[64:65, hh, :],
                                start=True, stop=True,
                            )
                        nc.vector.tensor_tensor(
                            attn_sb[0:64, e, :], u_sb[0:64, 0, :], pb[:, 0, :], MUL
                        )
                        bounce = work.tile([64, LQ], bf16, tag="bounce")
                        nc.vector.tensor_tensor(
                            bounce, u_sb[0:64, 1, :], pb[:, 1, :], MUL
                        )
                        nc.sync.dma_start(attn_sb[64:128, e, :], bounce)

                # --- fc_out: final^T = Wout @ attn^T + bout ---
                with tc.tile_pool(name="po", bufs=2, space="PSUM") as pop:
                    for oc in range(E // 128):
                        po = pop.tile([128, LQ], f32, tag="po")
                        for ec in range(E // 128):
                            nc.tensor.matmul(
                                po,
                                wout_sb[:, ec, 128 * oc : 128 * (oc + 1)],
                                attn_sb[:, ec, :],
                                start=(ec == 0), stop=(ec == E // 128 - 1),
                            )
                        o_sb = work.tile([128, LQ], f32, tag="o_sb")
                        nc.vector.tensor_tensor(
                            o_sb, po,
                            bias_sb[:, oc : oc + 1].to_broadcast((128, LQ)),
                            ADD,
                        )
                        nc.sync.dma_start(
                            outT[128 * oc : 128 * (oc + 1), :], o_sb
                        )

    nc.compile()
    return nc


def shard_inputs(values, keys, query, Wv, Wk, Wq, Wout, bout):
    f = np.float32
    values = np.asarray(values, dtype=f)
    keys = np.asarray(keys, dtype=f)
    query = np.asarray(query, dtype=f)
    Wv, Wk, Wq, Wout, bout = (np.asarray(x, dtype=f) for x in (Wv, Wk, Wq, Wout, bout))

    # fold projections on host
    Wc = (Wq.T @ Wk) / np.float32(np.sqrt(E))
    q4 = query.reshape(N, L, H, D) @ Wc          # Q'' per head
    v4 = values.reshape(N, L, H, D) @ Wv.T       # V' per head

    # K^T (embed-major), bf16
    kT = np.ascontiguousarray(keys.transpose(0, 2, 1)).astype(BF16)
    # Q''^T, bf16
    qT = np.ascontiguousarray(
        q4.reshape(N, L, E).transpose(0, 2, 1)
    ).astype(BF16)
    # packed V': [n, pair, token%128, chunk, 130] = [V'_h0 | 1 | V'_h1 | 1]
    vp = np.ones((N, NPAIR, 128, NCHUNK, 130), dtype=f)
    # v4 -> [n, chunk, part, pair, head%2, d]
    v6 = v4.reshape(N, NCHUNK, 128, NPAIR, 2, D)
    vp[:, :, :, :, 0:64] = v6[:, :, :, :, 0].transpose(0, 3, 2, 1, 4)
    vp[:, :, :, :, 65:129] = v6[:, :, :, :, 1].transpose(0, 3, 2, 1, 4)
    vp = vp.astype(BF16)

    wout_r = np.ascontiguousarray(
        Wout.T.reshape(E // 128, 128, E).transpose(1, 0, 2)
    ).astype(BF16)
    bias2 = np.ascontiguousarray(bout.reshape(E // 128, 128).T, dtype=f)
    ones = np.ones((128, 128), dtype=f)

    in_maps = []
    for c in range(NCORES):
        n, qb = c // NQBLK, c % NQBLK
        in_maps.append({
            "kT": kT[n],
            "qT": np.ascontiguousarray(qT[n, :, qb * LQ : (qb + 1) * LQ]),
            "vp": vp[n],
            "wout": wout_r,
            "bias": bias2,
            "ones": ones,
        })
    return in_maps


def unshard(results):
    out = np.empty((N, L, E), dtype=np.float32)
    for c, r in enumerate(results):
        n, qb = c // NQBLK, c % NQBLK
        out[n, qb * LQ : (qb + 1) * LQ, :] = np.asarray(r["outT"]).T
    return out


def run_spmd(in_maps, **kwargs):
    from concourse.bass_utils import run_bass_kernel_spmd

    nc = build_nc()
    res = run_bass_kernel_spmd(nc, in_maps, core_ids=list(range(NCORES)), **kwargs)
    return nc, res


def kernel(**inputs):
    in_maps = shard_inputs(
        inputs["values"], inputs["keys"], inputs["query"],
        inputs["Wv"], inputs["Wk"], inputs["Wq"],
        inputs["Wout"], inputs["bout"],
    )
    _, res = run_spmd(in_maps)
    return unshard(res.results)


if __name__ == "__main__":
    rng = np.random.default_rng(0)
    ins = {
        "values": rng.standard_normal((N, L, E), dtype=np.float32),
        "keys": rng.standard_normal((N, L, E), dtype=np.float32),
        "query": rng.standard_normal((N, L, E), dtype=np.float32),
        "Wv": rng.standard_normal((D, D), dtype=np.float32) / 8,
        "Wk": rng.standard_normal((D, D), dtype=np.float32) / 8,
        "Wq": rng.standard_normal((D, D), dtype=np.float32) / 8,
        "Wout": rng.standard_normal((E, E), dtype=np.float32) / 32,
        "bout": rng.standard_normal((E,), dtype=np.float32) * 0.01,
    }
    out = kernel(**ins)
    print("out", out.shape, out.dtype, float(np.abs(out).max()))
